# revision 22
# baseline (speedup 1.0000x reference)
"""BERT embedding (token + position + type lookup, then LayerNorm) on 8 TRN2
NeuronCores.

Strategy (hardcoded for B=32, S=512, H=768, V=30522, TYPE_VOCAB=2):

- Data-parallel over batch: 4 sequences (2048 tokens) per core; the token
  table is replicated per core.
- Host-side preprocessing folds most of the math into the tables:
    * Every table row is pre-centered (row minus its row-mean) in f64.  The
      per-token mean of the embedding sum is the sum of the three row
      means, so the summed embedding becomes exactly mean-free -> no mean
      subtraction on device, and var = mean(x^2) exactly.
    * type_w row 0 is folded into the token table; the type contribution
      becomes token_type_id * diff  (diff = centered type row1 - row0).
- On device, per 256-token tile (tokens on partitions, H in free dim):
    * GPSIMD dma_gather fetches the 256 token rows from DRAM (int16 row
      indices staged in the queue-0 stripe layout the Q7 ucode reads).
      GPSIMD runs nothing else, so descriptor generation for tile k+1 is
      never blocked behind tile k's compute.
    * DVE adds the resident position rows and token_type*diff,
    * ACT Square+accum produces row sums of squares (rows are mean-free),
    * sqrt(var+eps) on ACT, reciprocal on DVE, scale-by-rstd on ACT,
    * result DMAs out via HWDGE (sync engine), 6KB contiguous per
      partition (tokens are ordered t = k*256 + 2p + j inside a tile).
- gamma/beta: the kernel is specialized at trace time.  If gamma==1 and
  beta==0 (checked on host) they are skipped; otherwise they are applied
  with DVE tensor ops against partition-broadcast tiles.
"""

import sys

for _p in ("/opt/trn_rl_repo", "/root/.axon_site/_ro/trn_rl_repo"):
    if _p not in sys.path:
        sys.path.append(_p)

import numpy as np

import concourse.bacc as bacc
import concourse.bass as bass
import concourse.tile as tile
from concourse import mybir
from concourse.bass_utils import run_bass_kernel_spmd

# Problem constants (hardcoded per the self-contained-kernel contract).
B, S, H = 32, 512, 768
VOCAB, TYPE_VOCAB, MAX_POS = 30522, 2, 512
EPS = 1e-5
N_CORES = 8
B_PER_CORE = B // N_CORES            # 4
T_PER_CORE = B_PER_CORE * S          # 2048 tokens
J = 2                                # tokens per partition per tile
TPT = 128 * J                        # 256 tokens per tile
NT = T_PER_CORE // TPT               # 8 tiles per core
NIW = TPT // 16                      # int16 index columns per tile (16)

F32 = mybir.dt.float32
I16 = mybir.dt.int16

_BUILD_CACHE = {}

# Token ordering inside a tile: SBUF slot (p, j) <-> flat token
# t = k*TPT + 2*p + j.  dma_gather writes list position i to slot
# (i%128, i//128), so list position i carries token k*TPT + perm(i),
# perm(i) = 2*(i%128) + i//128.
_PERM = (2 * (np.arange(TPT) % 128) + np.arange(TPT) // 128)


def _build(affine: bool, stage: str = "full", nt: int = NT):
    """stage: 'g' gather only, 'gp' +pos, 'gpt' +type, 'full' everything."""
    nc = bacc.Bacc("TRN2")

    ctab = nc.dram_tensor("ctab", [VOCAB, H], F32, kind="ExternalInput")
    posc = nc.dram_tensor("posc", [S, H], F32, kind="ExternalInput")
    diff = nc.dram_tensor("diff", [H], F32, kind="ExternalInput")
    idx = nc.dram_tensor("idx", [128, NT, NIW], I16, kind="ExternalInput")
    ttf = nc.dram_tensor("ttf", [128, NT, J], F32, kind="ExternalInput")
    if affine:
        gamma = nc.dram_tensor("gamma", [H], F32, kind="ExternalInput")
        beta = nc.dram_tensor("beta", [H], F32, kind="ExternalInput")
    out = nc.dram_tensor("out", [T_PER_CORE, H], F32, kind="ExternalOutput")

    def bcast128(ap):
        return bass.AP(tensor=ap.tensor, offset=ap.offset,
                       ap=[[0, 128]] + list(ap.ap))

    with tile.TileContext(nc) as tc:
        with (
            tc.tile_pool(name="singles", bufs=1) as singles,
            tc.tile_pool(name="gp", bufs=4) as g_pool,
            tc.tile_pool(name="tmpp", bufs=3) as tmp_pool,
            tc.tile_pool(name="sqp", bufs=2) as sq_pool,
            tc.tile_pool(name="outp", bufs=3) as out_pool,
            tc.tile_pool(name="small", bufs=4) as small_pool,
        ):
            # Gather indices first so the gathers start immediately
            # (host already stripes/replicates them for the Q7 ucode).
            # SWDGE path: separate rings from the bulk HWDGE preloads, so
            # the tiny idx packets don't interleave behind 1.5MB of pos.
            idx_res = singles.tile([128, NT, NIW], I16)
            nc.gpsimd.dma_start(out=idx_res[:], in_=idx[:, :, :])
            # Position rows, pre-centered, resident. s = m*TPT + 2p + j.
            tiles_per_seq = S // TPT  # 2
            pos_res = singles.tile([128, tiles_per_seq, J, H], F32)
            for m in range(tiles_per_seq):
                nc.sync.dma_start(
                    out=pos_res[:, m, :, :],
                    in_=posc[m * TPT:(m + 1) * TPT, :].rearrange(
                        "(p j) h -> p j h", j=J
                    ),
                )
            # token-type as f32 scalars, [p, k, j]
            ttf_res = singles.tile([128, NT, J], F32)
            nc.gpsimd.dma_start(out=ttf_res[:], in_=ttf[:, :, :])
            # centered type diff row broadcast to all partitions
            diff_res = singles.tile([128, H], F32)
            nc.gpsimd.dma_start(out=diff_res[:], in_=bcast128(diff[:]))
            eps_t = singles.tile([128, 1], F32)
            nc.vector.memset(eps_t[:], EPS)
            if affine:
                gamma_res = singles.tile([128, H], F32)
                nc.sync.dma_start(out=gamma_res[:], in_=bcast128(gamma[:]))
                beta_res = singles.tile([128, H], F32)
                nc.sync.dma_start(out=beta_res[:], in_=bcast128(beta[:]))

            # out rows (k, p, j) with j fastest -> per-partition 6KB chunks
            out_t = out[:, :].rearrange("(k p j) h -> k p (j h)", p=128, j=J)

            for k in range(nt):
                m = k % tiles_per_seq
                g = g_pool.tile([128, J, H], F32)
                # token rows: list position i -> g[i%128, i//128, :]
                nc.gpsimd.dma_gather(g[:], ctab[:, :], idx_res[:, k, :],
                                     TPT, TPT, H)
                if stage == "g":
                    nc.sync.dma_start(out=out_t[k],
                                      in_=g[:].rearrange("p j h -> p (j h)"))
                    continue
                # g += pos
                nc.vector.tensor_add(out=g[:], in0=g[:],
                                     in1=pos_res[:, m, :, :])
                if stage == "gp":
                    nc.sync.dma_start(out=out_t[k],
                                      in_=g[:].rearrange("p j h -> p (j h)"))
                    continue
                # tmp = token_type * diff  (DVE tensor_scalar, 2x mode)
                tmp = tmp_pool.tile([128, J, H], F32)
                for j in range(J):
                    nc.vector.tensor_scalar_mul(
                        out=tmp[:, j, :], in0=diff_res[:],
                        scalar1=ttf_res[:, k, j:j + 1])
                # g += tmp
                nc.vector.tensor_add(out=g[:], in0=g[:], in1=tmp[:])
                if stage == "gpt":
                    nc.sync.dma_start(out=out_t[k],
                                      in_=g[:].rearrange("p j h -> p (j h)"))
                    continue
                # row sums of squares on ACT (Square is exact; rows are
                # mean-free so var = ssq/H).  NOTE: tensor_tensor_reduce
                # crashes the device on this ucode build — do not use it.
                sq = sq_pool.tile([128, J, H], F32)
                ssq = small_pool.tile([128, J], F32)
                for j in range(J):
                    nc.scalar.activation(
                        out=sq[:, j, :],
                        in_=g[:, j, :],
                        func=mybir.ActivationFunctionType.Square,
                        accum_out=ssq[:, j:j + 1],
                    )
                # rstd = 1/sqrt(ssq/H + eps)
                rstd = small_pool.tile([128, J], F32)
                nc.scalar.activation(
                    out=rstd[:],
                    in_=ssq[:],
                    func=mybir.ActivationFunctionType.Sqrt,
                    bias=eps_t[:, :1],
                    scale=1.0 / H,
                )
                nc.vector.reciprocal(out=rstd[:], in_=rstd[:])

                o = out_pool.tile([128, J, H], F32)
                for j in range(J):
                    nc.scalar.mul(out=o[:, j, :], in_=g[:, j, :],
                                  mul=rstd[:, j:j + 1])
                    if affine:
                        nc.vector.tensor_mul(out=o[:, j, :], in0=o[:, j, :],
                                             in1=gamma_res[:])
                        nc.vector.tensor_add(out=o[:, j, :], in0=o[:, j, :],
                                             in1=beta_res[:])
                nc.sync.dma_start(out=out_t[k],
                                  in_=o[:].rearrange("p j h -> p (j h)"))

    nc.compile()
    return nc


def _get_nc(affine: bool):
    key = ("v12", affine)
    if key not in _BUILD_CACHE:
        _BUILD_CACHE[key] = _build(affine)
    return _BUILD_CACHE[key]


def _host_prep(input_ids, token_type_ids, tok_w, pos_w, type_w):
    """Returns (ctab, posc, diff, per-core idx int16 stacks, per-core ttf)."""
    tok64 = tok_w.astype(np.float64)
    tokc = tok64 - tok64.mean(axis=1, keepdims=True)
    ty64 = type_w.astype(np.float64)
    tyc = ty64 - ty64.mean(axis=1, keepdims=True)
    pos64 = pos_w.astype(np.float64)
    posc = (pos64 - pos64.mean(axis=1, keepdims=True)).astype(np.float32)
    ctab = (tokc + tyc[0]).astype(np.float32)
    diff = (tyc[1] - tyc[0]).astype(np.float32)

    ids = input_ids.astype(np.int64)          # [B, S]
    tts = token_type_ids.astype(np.int64)     # [B, S]

    idx_cores, ttf_cores = [], []
    for c in range(N_CORES):
        flat = ids[c * B_PER_CORE:(c + 1) * B_PER_CORE].reshape(-1)  # [2048]
        # list[i] of tile k = token k*TPT + _PERM[i]; the ucode reads list
        # position i from [16*b + i%16, i//16] (stripe b replicated so any
        # queue stripe and CoreSim agree).
        lists = flat.reshape(NT, TPT)[:, _PERM]              # [NT, TPT]
        per_tile = lists.reshape(NT, NIW, 16).transpose(0, 2, 1)  # [NT,16,NIW]
        idx16 = np.broadcast_to(
            per_tile[:, None, :, :], (NT, 8, 16, NIW)
        ).reshape(NT, 128, NIW).transpose(1, 0, 2).astype(np.int16)  # [128,NT,NIW]
        idx_cores.append(np.ascontiguousarray(idx16))
        tflat = tts[c * B_PER_CORE:(c + 1) * B_PER_CORE].reshape(-1)
        # ttf[p, k, j] = type of token k*TPT + 2p + j
        ttfv = tflat.reshape(NT, 128, J).transpose(1, 0, 2).astype(np.float32)
        ttf_cores.append(np.ascontiguousarray(ttfv))
    return ctab, posc, diff, idx_cores, ttf_cores


def kernel(input_ids, token_type_ids, tok_w, pos_w, type_w, gamma, beta):
    input_ids = np.asarray(input_ids)
    token_type_ids = np.asarray(token_type_ids)
    tok_w = np.asarray(tok_w, dtype=np.float32)
    pos_w = np.asarray(pos_w, dtype=np.float32)
    type_w = np.asarray(type_w, dtype=np.float32)
    gamma = np.asarray(gamma, dtype=np.float32)
    beta = np.asarray(beta, dtype=np.float32)

    affine = not (np.all(gamma == 1.0) and np.all(beta == 0.0))
    ctab, posc, diff, idx_cores, ttf_cores = _host_prep(
        input_ids, token_type_ids, tok_w, pos_w, type_w
    )

    in_maps = []
    for c in range(N_CORES):
        m = {
            "ctab": ctab,
            "posc": posc,
            "diff": diff,
            "idx": idx_cores[c],
            "ttf": ttf_cores[c],
        }
        if affine:
            m["gamma"] = gamma
            m["beta"] = beta
        in_maps.append(m)

    nc = _get_nc(affine)
    res = run_bass_kernel_spmd(nc, in_maps, list(range(N_CORES)))
    kernel.last_results = res

    out = np.empty((B, S, H), dtype=np.float32)
    for c in range(N_CORES):
        out[c * B_PER_CORE:(c + 1) * B_PER_CORE] = (
            res.results[c]["out"].reshape(B_PER_CORE, S, H)
        )
    return out
